# revision 18
# baseline (speedup 1.0000x reference)
"""GPT-2 (6L, D=768, H=12, B=2, T=1024, V=50257) forward pass on 8 trn2 cores.

Sharding: tokens 2048 -> 256/core (cores 0-3 = batch 0, 4-7 = batch 1).
Per-layer split K/V AllGathers (bf16) within each 4-core batch group (K-AG
issues early; V-AG hides behind the score/exp pipeline); every core computes
full-kv attention for its 256 queries with a multiplicative {0,1} causal
mask applied after exp (SPMD-uniform work). Logits:
vocab-sharded (6656 padded cols/core) against an AllGathered bf16 final
hidden state; host concatenates shards.

Precision: weights + most activations bf16 (fp32 PSUM accumulate), the
residual stream fp32.
"""

import sys
from contextlib import ExitStack

import numpy as np

sys.path.insert(0, "/opt/trn_rl_repo")

import ml_dtypes
import concourse.bass as bass
import concourse.tile as tile
from concourse import bacc, mybir
from concourse.bass_utils import run_bass_kernel_spmd

F32 = mybir.dt.float32
F32R = mybir.dt.float32r
BF16 = mybir.dt.bfloat16
FP8 = mybir.dt.float8e4
AF = mybir.ActivationFunctionType
ALU = mybir.AluOpType

NP_BF16 = ml_dtypes.bfloat16
NP_FP8 = ml_dtypes.float8_e4m3

L, D, V, B, T, H, HD = 6, 768, 50257, 2, 1024, 12, 64
NTOK = 256           # tokens per core
NC = 8               # cores
KT = D // 128        # 6 feature tiles
VSHARD = 6656        # padded vocab per core (13 * 512); 8*6656 = 53248
VT = VSHARD // 512   # 13
TT = (B * T) // 128  # 16 token tiles of the full sequence

_CACHE = {}


def _r(x):
    return x


def build_nc(debug=False):
    nc = bacc.Bacc("TRN2", target_bir_lowering=False, debug=False, num_devices=NC)

    # ---- per-core inputs ----
    x0T = nc.dram_tensor("x0T", [D, NTOK], F32R, kind="ExternalInput")
    onesd = nc.dram_tensor("onesd", [128, 1], F32R, kind="ExternalInput")
    ones8 = nc.dram_tensor("ones8", [128, 12], BF16, kind="ExternalInput")
    mask01 = nc.dram_tensor("mask01", [128, 8, NTOK], BF16, kind="ExternalInput")
    wteT = nc.dram_tensor("wteT", [KT, VT, 128, 512], BF16, kind="ExternalInput")
    # ---- replicated weights (bf16) ----
    wqk = nc.dram_tensor("wqk", [L, 12, 128, KT, 128], BF16, kind="ExternalInput")
    wv = nc.dram_tensor("wv", [L, 2, 128, KT, 384], BF16, kind="ExternalInput")
    wproj = nc.dram_tensor("wproj", [L, KT, 128, KT, 128], BF16, kind="ExternalInput")
    wfc = nc.dram_tensor("wfc", [L, 24, 128, KT, 128], BF16, kind="ExternalInput")
    wfc2 = nc.dram_tensor("wfc2", [L, KT, 128, 24, 128], BF16, kind="ExternalInput")
    b_qkv = nc.dram_tensor("b_qkv", [L, 128, 12], F32, kind="ExternalInput")
    b_v = nc.dram_tensor("b_v", [L, 768], F32, kind="ExternalInput")
    b_proj = nc.dram_tensor("b_proj", [L, 128, KT], F32, kind="ExternalInput")
    b_fc = nc.dram_tensor("b_fc", [L, 128, 24], F32, kind="ExternalInput")
    b_fc2 = nc.dram_tensor("b_fc2", [L, 128, KT], F32, kind="ExternalInput")
    s_ln1 = nc.dram_tensor("s_ln1", [L, 128, KT], F32, kind="ExternalInput")
    bi_ln1 = nc.dram_tensor("bi_ln1", [L, 128, KT], F32, kind="ExternalInput")
    s_ln2 = nc.dram_tensor("s_ln2", [L, 128, KT], F32, kind="ExternalInput")
    bi_ln2 = nc.dram_tensor("bi_ln2", [L, 128, KT], F32, kind="ExternalInput")
    s_lnf = nc.dram_tensor("s_lnf", [128, KT], F32, kind="ExternalInput")
    bi_lnf = nc.dram_tensor("bi_lnf", [128, KT], F32, kind="ExternalInput")
    # ---- output ----
    out = nc.dram_tensor("out", [B * T, VSHARD], F32, kind="ExternalOutput")

    # ---- collective bounce buffers ----
    # bf16 payloads, but declared f32 (bf16-dtype collectives crash NRT);
    # SBUF APs are bitcast at the DMA boundaries.
    KVSZ = D * NTOK      # bf16 elems for k (and again for v)
    HKV = KVSZ // 2      # same bytes in f32 elems
    k_in_t = nc.dram_tensor("k_in_t", [HKV], F32)
    v_in_t = nc.dram_tensor("v_in_t", [HKV], F32)
    k_out = nc.dram_tensor("k_out", [4 * HKV], F32)
    v_out = nc.dram_tensor("v_out", [4 * HKV], F32)
    xf_in = nc.dram_tensor("xf_in", [HKV], F32)
    xf_out = nc.dram_tensor("xf_out", [NC * HKV], F32, addr_space="Shared")
    kv_groups = [[0, 1, 2, 3], [4, 5, 6, 7]]

    with tile.TileContext(nc) as tc, ExitStack() as ctx:
        const = ctx.enter_context(tc.tile_pool(name="const", bufs=1))
        ones = const.tile([128, 1], F32R)
        nc.sync.dma_start(out=ones, in_=onesd.ap())
        eps = const.tile([1, 1], F32)
        nc.vector.memset(eps, 1e-5)
        mask_sb = const.tile([128, 8, NTOK], BF16)
        nc.sync.dma_start(out=mask_sb, in_=mask01.ap())
        scl = const.tile([128, 4 * L + 2, KT], F32)  # ln scales/biases
        for l in range(L):
            nc.sync.dma_start(out=scl[:, 4 * l + 0, :], in_=s_ln1[l])
            nc.sync.dma_start(out=scl[:, 4 * l + 1, :], in_=bi_ln1[l])
            nc.sync.dma_start(out=scl[:, 4 * l + 2, :], in_=s_ln2[l])
            nc.sync.dma_start(out=scl[:, 4 * l + 3, :], in_=bi_ln2[l])
        nc.sync.dma_start(out=scl[:, 4 * L + 0, :], in_=s_lnf.ap())
        nc.sync.dma_start(out=scl[:, 4 * L + 1, :], in_=bi_lnf.ap())
        bias_sb = const.tile([128, L, 12 + KT + 24 + KT], F32)
        for l in range(L):
            nc.sync.dma_start(out=bias_sb[:, l, 0:12], in_=b_qkv[l])
            nc.sync.dma_start(out=bias_sb[:, l, 12 : 12 + KT], in_=b_proj[l])
            nc.sync.dma_start(out=bias_sb[:, l, 18:42], in_=b_fc[l])
            nc.sync.dma_start(out=bias_sb[:, l, 42:48], in_=b_fc2[l])

        with ExitStack() as body:
            resid = body.enter_context(tc.tile_pool(name="resid", bufs=2))
            lnp = body.enter_context(tc.tile_pool(name="lnp", bufs=1))
            qkvp = body.enter_context(tc.tile_pool(name="qkvp", bufs=1))
            kvp = body.enter_context(tc.tile_pool(name="kvp", bufs=1))
            wpool = body.enter_context(tc.tile_pool(name="wpool", bufs=1))
            att = body.enter_context(tc.tile_pool(name="att", bufs=1))
            yp = body.enter_context(tc.tile_pool(name="yp", bufs=1))
            hp = body.enter_context(tc.tile_pool(name="hp", bufs=1))
            stat = body.enter_context(tc.tile_pool(name="stat", bufs=1))
            ps_mm = body.enter_context(tc.tile_pool(name="ps_mm", bufs=2, space="PSUM"))
            ps_s = body.enter_context(tc.tile_pool(name="ps_s", bufs=2, space="PSUM"))
            ps_av = body.enter_context(tc.tile_pool(name="ps_av", bufs=2, space="PSUM"))

            dbg_n = [0]

            def dump(ap):
                if not debug:
                    return
                s = dbg_n[0]
                dbg_n[0] += 1
                p = ap.shape[0]
                t = stat.tile([128, 256], F32, name=f"dbg{s}", tag=f"dbg{s}")
                nc.vector.tensor_scalar(out=t[0:p, 0 : ap.free_size()], in0=ap,
                                        scalar1=1.0, scalar2=None, op0=ALU.mult)
                nc.sync.dma_start(
                    out=out.ap()[(s % 16) * 128 : (s % 16) * 128 + p,
                                 (s // 16) * 256 : (s // 16) * 256 + ap.free_size()],
                    in_=t[0:p, 0 : ap.free_size()])

            # persistent gathered-KV tiles; ones column of v loaded once
            k_sb = [kvp.tile([128, T], BF16, name=f"k{j}", tag=f"k{j}") for j in range(KT)]
            v_sb = [kvp.tile([128, 12, 66], BF16, name=f"v{j}", tag=f"v{j}") for j in range(8)]
            for j in range(8):
                nc.sync.dma_start(out=v_sb[j][:, :, 64:65], in_=ones8.ap())

            x_tiles = []
            for j in range(KT):
                xt = resid.tile([128, NTOK], F32R, name=f"x{j}", tag=f"x{j}")
                nc.sync.dma_start(out=xt, in_=x0T[j * 128 : (j + 1) * 128, :])
                x_tiles.append(xt)

            def layernorm(xs, s_col, b_col):
                st = ps_av.tile([65, NTOK], F32, name="st", tag="av")
                st2 = ps_av.tile([65, NTOK], F32, name="st2", tag="av")
                sq = []
                for j in range(KT):
                    sqt = lnp.tile([128, NTOK], F32R, name=f"sq{j}", tag=f"sq{j}")
                    nc.vector.tensor_mul(sqt, xs[j], xs[j])
                    sq.append(sqt)
                for j in range(KT):
                    nc.tensor.matmul(st[0:1, :], _r(ones), _r(xs[j]),
                                     start=(j == 0), stop=(j == KT - 1))
                for j in range(KT):
                    nc.tensor.matmul(st2[0:1, :], _r(ones), _r(sq[j]),
                                     start=(j == 0), stop=(j == KT - 1))
                mu_t = stat.tile([1, NTOK], F32, name="mu_t", tag="mu_t")
                rs_t = stat.tile([1, NTOK], F32, name="rs_t", tag="rs_t")
                mu = mu_t[0:1, :]
                rs = rs_t[0:1, :]
                nc.scalar.mul(mu, st[0:1, :], 1.0 / D)
                musq = stat.tile([1, NTOK], F32, name="musq", tag="musq")
                nc.vector.tensor_mul(musq, mu, mu)
                var = stat.tile([1, NTOK], F32, name="var", tag="var")
                nc.vector.tensor_scalar(out=var, in0=st2[0:1, :], scalar1=1.0 / D,
                                        scalar2=None, op0=ALU.mult)
                nc.vector.tensor_sub(var, var, musq)
                # rs = 1/sqrt(var+eps) = exp(-0.5*ln(var+eps)); ln+exp share
                # one ACT table set (unlike sqrt), avoiding per-LN set loads
                lnv = stat.tile([1, NTOK], F32, name="lnv", tag="lnv")
                nc.scalar.activation(lnv, var, AF.Ln, bias=eps)
                nc.scalar.activation(rs, lnv, AF.Exp, scale=-0.5)
                bc = stat.tile([128, 2, NTOK], F32, name="bc", tag="bc")
                nc.gpsimd.partition_broadcast(bc[:, 0, :], mu)
                nc.gpsimd.partition_broadcast(bc[:, 1, :], rs)
                outs = []
                for j in range(KT):
                    t = lnp.tile([128, NTOK], F32, name=f"lt{j}", tag=f"lt{j}")
                    nc.vector.tensor_sub(t, xs[j], bc[:, 0, :])
                    ot = lnp.tile([128, NTOK], BF16, name=f"ln{j}", tag=f"ln{j}")
                    nc.vector.scalar_tensor_tensor(
                        out=ot, in0=t, scalar=s_col[:, j : j + 1], in1=bc[:, 1, :],
                        op0=ALU.mult, op1=ALU.mult)
                    # note: (x-mu)*s*rs then +b
                    nc.vector.tensor_scalar(out=ot, in0=ot,
                                            scalar1=b_col[:, j : j + 1],
                                            scalar2=None, op0=ALU.add)
                    outs.append(ot)
                return outs

            k_in = k_in_t.ap().rearrange("(p t) -> p t", p=D)   # [768, 128] f32
            v_in = v_in_t.ap().rearrange("(t d) -> t d", t=NTOK)  # [256, 384] f32

            for l in range(L):
                if l == 0:
                    dump(x_tiles[0])          # slot 0
                ln1 = layernorm(x_tiles, scl[:, 4 * l + 0, :], scl[:, 4 * l + 1, :])
                if l == 0:
                    dump(ln1[0])              # slot 1

                # --- K part of QKV (feature-major [768, 256]) -> kv_in ---
                for ot in range(6, 12):
                    wt = wpool.tile([128, KT, 128], BF16, name="wqk", tag="wqk", bufs=3)
                    nc.sync.dma_start(out=wt, in_=wqk[l, ot])
                    ps = ps_mm.tile([128, 384], F32, name="mm", tag="mm")
                    for j in range(KT):
                        nc.tensor.matmul(ps[:, 0:NTOK], _r(wt[:, j, :]), _r(ln1[j]),
                                         start=(j == 0), stop=(j == KT - 1))
                    sb = qkvp.tile([128, NTOK], BF16, name=f"k{ot}", tag=f"qk{ot}")
                    nc.vector.tensor_scalar_add(sb, ps[:, 0:NTOK],
                                                bias_sb[:, l, ot : ot + 1])
                    nc.sync.dma_start(
                        out=k_in[(ot - 6) * 128 : (ot - 5) * 128, :],
                        in_=sb.bitcast(F32))
                    if l == 0 and ot == 6:
                        dump(sb)              # slot 2

                nc.gpsimd.collective_compute(
                    "AllGather", ALU.bypass, replica_groups=kv_groups,
                    ins=[k_in_t.ap()], outs=[k_out.ap()],
                )

                # --- V part: token-major [256, 768] -> kv_in (overlaps K-AG) ---
                bv_sb = wpool.tile([128, 768], F32, name="bv", tag="bv")
                bvl = b_v.ap()[l]
                nc.sync.dma_start(
                    out=bv_sb,
                    in_=bass.AP(tensor=bvl.tensor, offset=bvl.offset,
                                ap=[[0, 128]] + list(bvl.ap)),
                )
                vloc = [qkvp.tile([128, 768], BF16, name=f"vloc{tt}", tag=f"vloc{tt}")
                        for tt in range(2)]
                for oh in range(2):
                    wt = wpool.tile([128, KT, 384], BF16, name="wvt", tag="wvt", bufs=2)
                    nc.sync.dma_start(out=wt, in_=wv[l, oh])
                    for tt in range(2):
                        ps = ps_mm.tile([128, 384], F32, name="mmv", tag="mm")
                        for j in range(KT):
                            nc.tensor.matmul(
                                ps, _r(ln1[j][:, tt * 128 : (tt + 1) * 128]),
                                _r(wt[:, j, :]),
                                start=(j == 0), stop=(j == KT - 1))
                        nc.vector.tensor_add(
                            vloc[tt][:, oh * 384 : (oh + 1) * 384], ps,
                            bv_sb[:, oh * 384 : (oh + 1) * 384])
                for tt in range(2):
                    nc.sync.dma_start(
                        out=v_in[tt * 128 : (tt + 1) * 128, :],
                        in_=vloc[tt].bitcast(F32))
                if l == 0:
                    dump(vloc[0][:, 0:256])   # slot 3

                nc.gpsimd.collective_compute(
                    "AllGather", ALU.bypass, replica_groups=kv_groups,
                    ins=[v_in_t.ap()], outs=[v_out.ap()],
                )

                # --- Q part (overlaps the AllGather) ---
                qT = []
                for ot in range(6):
                    wt = wpool.tile([128, KT, 128], BF16, name="wqk", tag="wqk", bufs=3)
                    nc.sync.dma_start(out=wt, in_=wqk[l, ot])
                    ps = ps_mm.tile([128, 384], F32, name="mm", tag="mm")
                    for j in range(KT):
                        nc.tensor.matmul(ps[:, 0:NTOK], _r(wt[:, j, :]), _r(ln1[j]),
                                         start=(j == 0), stop=(j == KT - 1))
                    sb = qkvp.tile([128, NTOK], BF16, name=f"q{ot}", tag=f"qk{ot}")
                    nc.vector.tensor_scalar_add(sb, ps[:, 0:NTOK],
                                                bias_sb[:, l, ot : ot + 1])
                    qT.append(sb)
                    if l == 0 and ot == 0:
                        dump(sb)              # slot 4

                # --- load gathered K (feature-major [768, 1024]) and V ---
                ko = k_out.ap()
                vo = v_out.ap()
                for r in range(4):
                    k_r = ko[r * HKV : (r + 1) * HKV].rearrange("(p t) -> p t", p=D)
                    v_r = vo[r * HKV : (r + 1) * HKV].rearrange(
                        "(t h d) -> t h d", t=NTOK, h=12)
                    for j in range(KT):
                        nc.sync.dma_start(
                            out=k_sb[j][:, r * NTOK : (r + 1) * NTOK].bitcast(F32),
                            in_=k_r[j * 128 : (j + 1) * 128, :])
                    for tt in range(2):
                        nc.sync.dma_start(
                            out=v_sb[2 * r + tt][:, :, 0:64].bitcast(F32),
                            in_=v_r[tt * 128 : (tt + 1) * 128])
                if l == 0:
                    dump(k_sb[0][:, 0:256])   # slot 5
                    dump(v_sb[0][:, 0, 0:65])    # slot 6 (65 cols)

                # --- attention per head; scores in 4-kt chunks, batched exp ---
                yT = [yp.tile([128, NTOK], BF16, name=f"y{j}", tag=f"y{j}")
                      for j in range(KT)]
                for h in range(H):
                    p0 = 64 * (h % 2)
                    q_ap = qT[h // 2][p0 : p0 + 64, :]
                    psy = ps_av.tile([65, NTOK], F32, name="av", tag="av")
                    for c in range(2):
                        pss = ps_s.tile([128, 4, NTOK], F32, name="s", tag="s")
                        for k4 in range(4):
                            kt = 4 * c + k4
                            nc.tensor.matmul(
                                pss[:, k4, :],
                                _r(k_sb[h // 2][p0 : p0 + 64,
                                                kt * 128 : (kt + 1) * 128]),
                                _r(q_ap), start=True, stop=True)
                        eb = att.tile([128, 4, NTOK], BF16, name="eb", tag="eb",
                                      bufs=2)
                        nc.scalar.activation(eb, pss, AF.Exp, scale=0.125)
                        e8 = att.tile([128, 4, NTOK], BF16, name="e8", tag="e8",
                                      bufs=2)
                        nc.vector.tensor_mul(e8, eb, mask_sb[:, 4 * c : 4 * c + 4, :])
                        if l == 0 and h == 0 and c == 0:
                            dump(eb[:, 0, :])   # slot 7
                            dump(e8[:, 0, :])   # slot 8
                        for k4 in range(4):
                            kt = 4 * c + k4
                            nc.tensor.matmul(
                                psy, _r(v_sb[kt][:, h, 0:65]), _r(e8[:, k4, :]),
                                start=(kt == 0), stop=(kt == 7))
                    if l == 0 and h == 0:
                        dump(psy)             # slot 9 (65 cols x... 256)
                    den = stat.tile([1, NTOK], F32, name="den", tag="den")
                    nc.vector.tensor_scalar(out=den, in0=psy[64:65, :], scalar1=1.0,
                                            scalar2=None, op0=ALU.mult)
                    rec = stat.tile([1, NTOK], F32, name="rec", tag="rec")
                    nc.vector.reciprocal_approx_fast(rec, den)
                    rb = stat.tile([64, NTOK], F32, name="rb", tag="rb")
                    nc.gpsimd.partition_broadcast(rb, rec)
                    nc.vector.tensor_mul(yT[h // 2][p0 : p0 + 64, :],
                                         psy[0:64, :], rb)

                if l == 0:
                    dump(yT[0])               # slot 10
                # --- proj + residual ---
                x2_tiles = []
                for ot in range(KT):
                    wt = wpool.tile([128, KT, 128], BF16, name="wp", tag="wp", bufs=3)
                    nc.sync.dma_start(out=wt, in_=wproj[l, ot])
                    ps = ps_mm.tile([128, 384], F32, name="mm", tag="mm")
                    for j in range(KT):
                        nc.tensor.matmul(ps[:, 0:NTOK], _r(wt[:, j, :]), _r(yT[j]),
                                         start=(j == 0), stop=(j == KT - 1))
                    x2 = resid.tile([128, NTOK], F32R, name=f"x{ot}", tag=f"x{ot}")
                    nc.vector.scalar_tensor_tensor(
                        out=x2, in0=ps[:, 0:NTOK],
                        scalar=bias_sb[:, l, 12 + ot : 13 + ot],
                        in1=x_tiles[ot], op0=ALU.add, op1=ALU.add)
                    x2_tiles.append(x2)

                if l == 0:
                    dump(x2_tiles[0])         # slot 11
                # --- MLP ---
                ln2 = layernorm(x2_tiles, scl[:, 4 * l + 2, :], scl[:, 4 * l + 3, :])
                if l == 0:
                    dump(ln2[0])              # slot 12
                h_sb = []
                for ot in range(24):
                    wt = wpool.tile([128, KT, 128], BF16, name="wf", tag="wf", bufs=3)
                    nc.sync.dma_start(out=wt, in_=wfc[l, ot])
                    ps = ps_mm.tile([128, 384], F32, name="mm", tag="mm")
                    for j in range(KT):
                        nc.tensor.matmul(ps[:, 0:NTOK], _r(wt[:, j, :]), _r(ln2[j]),
                                         start=(j == 0), stop=(j == KT - 1))
                    hs = hp.tile([128, NTOK], BF16, name=f"h{ot}", tag=f"h{ot}")
                    nc.scalar.activation(hs, ps[:, 0:NTOK], AF.Gelu_apprx_tanh,
                                         bias=bias_sb[:, l, 18 + ot : 19 + ot])
                    h_sb.append(hs)
                x3_tiles = []
                for ot in range(KT):
                    wt = wpool.tile([128, 24, 128], BF16, name="w2", tag="w2", bufs=2)
                    nc.sync.dma_start(out=wt, in_=wfc2[l, ot])
                    ps = ps_mm.tile([128, 384], F32, name="mm", tag="mm")
                    for j in range(24):
                        nc.tensor.matmul(ps[:, 0:NTOK], _r(wt[:, j, :]), _r(h_sb[j]),
                                         start=(j == 0), stop=(j == 23))
                    x3 = resid.tile([128, NTOK], F32R, name=f"x{ot}", tag=f"x{ot}")
                    nc.vector.scalar_tensor_tensor(
                        out=x3, in0=ps[:, 0:NTOK],
                        scalar=bias_sb[:, l, 42 + ot : 43 + ot],
                        in1=x2_tiles[ot], op0=ALU.add, op1=ALU.add)
                    x3_tiles.append(x3)
                if l == 0:
                    dump(h_sb[0])             # slot 13
                    dump(x3_tiles[0])         # slot 14
                x_tiles = x3_tiles

            # --- final LN + AllGather of hidden state (bf16) ---
            lnf = layernorm(x_tiles, scl[:, 4 * L, :], scl[:, 4 * L + 1, :])
            xf_ap = xf_in.ap().rearrange("(p t) -> p t", p=D)  # [768, 128] f32
            for j in range(KT):
                nc.sync.dma_start(out=xf_ap[j * 128 : (j + 1) * 128, :],
                                  in_=lnf[j].bitcast(F32))
            nc.gpsimd.collective_compute(
                "AllGather", ALU.bypass, replica_groups=[list(range(NC))],
                ins=[xf_in.ap()], outs=[xf_out.ap()],
            )

        # --- logits: out[t, vshard] = xf.T @ wteT ---
        with ExitStack() as lg:
          if not debug:
            xfp = lg.enter_context(tc.tile_pool(name="xfp", bufs=1))
            wtep = lg.enter_context(tc.tile_pool(name="wtep", bufs=2))
            outp = lg.enter_context(tc.tile_pool(name="outp", bufs=4))
            ps_l = lg.enter_context(tc.tile_pool(name="ps_l", bufs=4, space="PSUM"))
            xfo = xf_out.ap()
            xf_sb = [xfp.tile([128, B * T], BF16, name=f"xf{j}", tag=f"xf{j}")
                     for j in range(KT)]
            for r in range(NC):
                x_r = xfo[r * HKV : (r + 1) * HKV].rearrange("(p t) -> p t", p=D)
                for j in range(KT):
                    nc.sync.dma_start(
                        out=xf_sb[j][:, r * NTOK : (r + 1) * NTOK].bitcast(F32),
                        in_=x_r[j * 128 : (j + 1) * 128, :])
            for vt in range(VT):
                wt_sb = [wtep.tile([128, 512], BF16, name=f"wte{j}", tag=f"wte{j}")
                         for j in range(KT)]
                for j in range(KT):
                    nc.sync.dma_start(out=wt_sb[j], in_=wteT[j, vt])
                for tt in range(TT):
                    ps = ps_l.tile([128, 512], F32, name="lg", tag="lg")
                    for j in range(KT):
                        nc.tensor.matmul(
                            ps, _r(xf_sb[j][:, tt * 128 : (tt + 1) * 128]),
                            _r(wt_sb[j]), start=(j == 0), stop=(j == KT - 1))
                    ot = outp.tile([128, 512], F32, name="out", tag="out")
                    nc.vector.tensor_scalar(out=ot, in0=ps, scalar1=1.0,
                                            scalar2=None, op0=ALU.mult)
                    nc.sync.dma_start(
                        out=out.ap()[tt * 128 : (tt + 1) * 128,
                                     vt * 512 : (vt + 1) * 512],
                        in_=ot)

    nc.compile()
    return nc


def prep_inputs(idx, wte, wpe, ln1_s, ln1_b, attn_w, attn_b, proj_w, proj_b,
                ln2_s, ln2_b, fc_w, fc_b, fc2_w, fc2_b, lnf_s, lnf_b):
    f = np.float32
    x0 = (wte[idx.reshape(-1)] + np.tile(wpe, (B, 1))).astype(f)  # [2048, 768]
    wte_pad = np.zeros((NC * VSHARD, D), f)
    wte_pad[:V] = wte
    shared = {
        "wqk": np.ascontiguousarray(
            attn_w[:, :, :1536].reshape(L, KT, 128, 12, 128)
            .transpose(0, 3, 2, 1, 4)).astype(NP_BF16),
        "wv": np.ascontiguousarray(
            attn_w[:, :, 1536:].reshape(L, KT, 128, 2, 384)
            .transpose(0, 3, 2, 1, 4)).astype(NP_BF16),
        "wproj": np.ascontiguousarray(
            proj_w.reshape(L, KT, 128, KT, 128).transpose(0, 3, 2, 1, 4)
        ).astype(NP_BF16),
        "wfc": np.ascontiguousarray(
            fc_w.reshape(L, KT, 128, 24, 128).transpose(0, 3, 2, 1, 4)
        ).astype(NP_BF16),
        "wfc2": np.ascontiguousarray(
            fc2_w.reshape(L, 24, 128, KT, 128).transpose(0, 3, 2, 1, 4)
        ).astype(NP_BF16),
        "b_qkv": np.ascontiguousarray(
            attn_b[:, :1536].reshape(L, 12, 128).transpose(0, 2, 1)).astype(f),
        "b_v": np.ascontiguousarray(attn_b[:, 1536:]).astype(f),
        "b_proj": np.ascontiguousarray(
            proj_b.reshape(L, KT, 128).transpose(0, 2, 1)).astype(f),
        "b_fc": np.ascontiguousarray(
            fc_b.reshape(L, 24, 128).transpose(0, 2, 1)).astype(f),
        "b_fc2": np.ascontiguousarray(
            fc2_b.reshape(L, KT, 128).transpose(0, 2, 1)).astype(f),
        "s_ln1": np.ascontiguousarray(
            ln1_s.reshape(L, KT, 128).transpose(0, 2, 1)).astype(f),
        "bi_ln1": np.ascontiguousarray(
            ln1_b.reshape(L, KT, 128).transpose(0, 2, 1)).astype(f),
        "s_ln2": np.ascontiguousarray(
            ln2_s.reshape(L, KT, 128).transpose(0, 2, 1)).astype(f),
        "bi_ln2": np.ascontiguousarray(
            ln2_b.reshape(L, KT, 128).transpose(0, 2, 1)).astype(f),
        "s_lnf": np.ascontiguousarray(lnf_s.reshape(KT, 128).T).astype(f),
        "bi_lnf": np.ascontiguousarray(lnf_b.reshape(KT, 128).T).astype(f),
    }
    in_maps = []
    tk = np.arange(T)[:, None]
    for c in range(NC):
        qs = NTOK * (c % 4)
        m = (tk <= qs + np.arange(NTOK)[None, :]).astype(f)  # [1024, 256] {0,1}
        m01 = np.ascontiguousarray(
            m.reshape(8, 128, NTOK).transpose(1, 0, 2)).astype(NP_BF16)
        wsh = wte_pad[c * VSHARD : (c + 1) * VSHARD]  # [6656, 768]
        wteT_t = np.ascontiguousarray(
            wsh.T.reshape(KT, 128, VT, 512).transpose(0, 2, 1, 3)).astype(NP_BF16)
        im = dict(shared)
        im["onesd"] = np.ones((128, 1), f)
        im["ones8"] = np.ones((128, 12), NP_BF16)
        im["x0T"] = np.ascontiguousarray(x0[c * NTOK : (c + 1) * NTOK].T)
        im["mask01"] = m01
        im["wteT"] = wteT_t
        in_maps.append(im)
    return in_maps


def kernel(**inputs):
    inputs = {k: np.asarray(v) for k, v in inputs.items()}
    in_maps = prep_inputs(**inputs)
    if "nc" not in _CACHE:
        _CACHE["nc"] = build_nc()
    res = run_bass_kernel_spmd(_CACHE["nc"], in_maps, list(range(NC)))
    shards = [res.results[c]["out"] for c in range(NC)]  # each [2048, 6656]
    full = np.concatenate(shards, axis=1)[:, :V]
    return np.ascontiguousarray(full.reshape(B, T, V))
